# revision 17
# baseline (speedup 1.0000x reference)
"""DeBut 2D conv (32,128,56,56) -> (32,256,56,56) on 8 axon TRN2 NeuronCores.

The butterfly product W3@W2@W1 composes to a block-diagonal (256,1152) matrix
with 32 blocks of (8 out x 36 in). Mapped onto conv weights, output channels
0-127 depend only on kernel positions 0-4 and channels 128-255 only on 4-8.
So the conv is 10 accumulating K=128 f32r matmuls per spatial strip instead
of 18 dense ones.

Per-core layout (batch sharded 4 images/core):
  x image -> SBUF padded (128, 58*58) float32r (zero ring = conv padding)
  7 strips of 8 output rows; moving operand = contiguous 464-col span of the
  padded image; psum (128, 8, 58); drain cols 0:56 with bias via DVE
  tensor_scalar_add (also the only sem lane PE waits on - f32r matmuls can
  encode just one sync wait).
"""
import numpy as np

# ---- problem constants (hardcoded; kernel.py must be self-contained) ----
B, C_IN, H, W = 32, 128, 56, 56
C_OUT = 256
KS = 3
N_CORES = 8
B_LOC = B // N_CORES          # 4 images per core
HP = H + 2                    # 58
PADDED = HP * HP + 8          # 3372, slack for junk-column overreads
STRIP_ROWS = 8
N_STRIPS = H // STRIP_ROWS    # 7
N_MM = STRIP_ROWS * HP        # 464 moving columns per matmul
N_OUT = STRIP_ROWS * W        # 448 valid columns per strip
R_SHAPES = [(768, 1152, 2, 3, 1), (512, 768, 2, 3, 2), (256, 512, 2, 4, 4)]
# (chunk, kpos) pairs: chunk0 -> kpos 0..4, chunk1 -> kpos 4..8
CHUNK_KPOS = [(0, k) for k in range(5)] + [(1, k) for k in range(4, 9)]

_RUNNER = None


def _compose_w(twiddle: np.ndarray) -> np.ndarray:
    """Compose butterfly factors into the dense (256, 1152) matrix (float64)."""
    W_full = None
    temp = 0
    for (osz, isz, row, col, diag) in R_SHAPES:
        npar = col * osz
        nb = isz // (col * diag)
        t = twiddle[temp:temp + npar].astype(np.float64)
        t = t.reshape(nb, diag, row, col).transpose(0, 2, 3, 1)  # (n, r, c, d)
        temp += npar
        Ws = np.zeros((osz, isz), np.float64)
        # out index n*row*diag + r*diag + d ; in index n*col*diag + c*diag + d
        for d in range(diag):
            for r in range(row):
                for c in range(col):
                    out_idx = np.arange(nb) * row * diag + r * diag + d
                    in_idx = np.arange(nb) * col * diag + c * diag + d
                    Ws[out_idx, in_idx] = t[:, r, c, d]
        W_full = Ws if W_full is None else Ws @ W_full
    return W_full  # (256, 1152)


def _build_nc(repeat: int = 1, trace_sim: bool = False, mode: str = 'full'):
    import concourse.bass as bass  # noqa: F401
    from concourse import bacc
    import concourse.mybir as mybir
    from concourse.tile import TileContext

    f32 = mybir.dt.float32
    f32r = mybir.dt.float32r

    nc = bacc.Bacc("TRN2", target_bir_lowering=False, debug=False,
                   num_devices=N_CORES)
    # xs/wts are declared float32r: same 4-byte layout (numpy float32 binds),
    # lets plain HWDGE DMAs feed the f32r matmuls with no cast pass.
    # xs arrives host-padded to 58x58 (zero ring included) so the in-DMA is
    # one contiguous 13.4KB run per partition.
    xs = nc.declare_dram_parameter("xs", [B_LOC, C_IN, HP * HP], f32r,
                                   isOutput=False)
    wts = nc.declare_dram_parameter("wts", [10, C_IN, 128], f32r, isOutput=False)
    biasT = nc.declare_dram_parameter("biasT", [128, 2], f32, isOutput=False)
    ys = nc.declare_dram_parameter("ys", [B_LOC, 2, 128, H * W], f32, isOutput=True)

    with TileContext(nc, trace_sim=trace_sim) as tc:
        with tc.tile_pool(name="sbuf", bufs=1) as cpool, \
             tc.tile_pool(name="outp", bufs=2) as opool, \
             tc.tile_pool(name="psum", bufs=6, space="PSUM") as ppool:
            # persistent padded-image slots; 8-col slack zeroed once below
            xpads = [cpool.tile([C_IN, PADDED], f32r, tag=f"xpad{i}",
                                name=f"xpad{i}")
                     for i in range(2)]
            zrow = cpool.tile([C_IN, 8], f32, tag="zrow")
            nc.vector.memset(zrow[:], 0.0)
            for xp in xpads:
                nc.vector.tensor_copy(xp[:, HP * HP:], zrow[:])   # slack
            # first image on the SP HWDGE queue, weights+bias on the ACT HWDGE
            # queue - they land in parallel
            nc.sync.dma_start(out=xpads[0][:, :HP * HP], in_=xs[0])
            wt_r = cpool.tile([C_IN, 10, 128], f32r, tag="wtr")
            nc.scalar.dma_start(out=wt_r[:], in_=wts.ap().rearrange("i c m -> c i m"))
            bias_sb = cpool.tile([128, 2], f32, tag="bias")
            nc.scalar.dma_start(out=bias_sb[:], in_=biasT.ap())

            # out-DMA split points: after strip 3 (cols 0:1792), after strip 5
            # (1792:2688), after strip 6 (2688:3136) - shrinks the tail
            OUT_SPLITS = {3: (0, 4 * N_OUT), 5: (4 * N_OUT, 6 * N_OUT),
                          6: (6 * N_OUT, 7 * N_OUT)}

            def body():
                for b in range(B_LOC):
                    if b > 0 or repeat > 1:  # image 0's DMA issued pre-loop (R=1)
                        nc.sync.dma_start(out=xpads[b % 2][:, :HP * HP],
                                          in_=xs[b])

                    out_sb = opool.tile([128, 2, H * W], f32, tag="outsb")
                    for s in range(N_STRIPS):
                        for chunk in range(2):
                            ps = ppool.tile([128, STRIP_ROWS, HP], f32, tag="ps")
                            for kidx in range(5):
                                _, k = CHUNK_KPOS[chunk * 5 + kidx]
                                di, dj = k // 3, k % 3
                                start = (STRIP_ROWS * s + di) * HP + dj
                                nc.tensor.matmul(
                                    ps[:],
                                    wt_r[:, chunk * 5 + kidx, :],
                                    xpads[b % 2][:, start:start + N_MM],
                                    start=(kidx == 0), stop=(kidx == 4),
                                )
                            nc.vector.tensor_scalar_add(
                                out_sb[:, chunk, s * N_OUT:(s + 1) * N_OUT]
                                .rearrange("p (r w) -> p r w", w=W),
                                ps[:, :, 0:W],
                                bias_sb[:, chunk:chunk + 1],
                            )
                        if s in OUT_SPLITS:
                            lo, hi = OUT_SPLITS[s]
                            nc.scalar.dma_start(
                                out=ys[b, :, :, lo:hi]
                                .rearrange("c2 p hw -> p c2 hw"),
                                in_=out_sb[:, :, lo:hi])

            if repeat == 1:
                body()
            else:
                with tc.For_i(0, repeat, 1):
                    body()
    nc.finalize()
    return nc


def _get_runner(repeat: int = 1):
    global _RUNNER
    if _RUNNER is None or _RUNNER[0] != repeat:
        from bass_exec_inline import BassRunner
        nc = _build_nc(repeat)
        _RUNNER = (repeat, BassRunner(nc, n_cores=N_CORES))
    return _RUNNER[1]


def _prep_params(twiddle: np.ndarray, bias: np.ndarray):
    W_dense = _compose_w(np.asarray(twiddle))
    wts = np.empty((10, C_IN, 128), np.float32)
    for i, (chunk, k) in enumerate(CHUNK_KPOS):
        blk = W_dense[chunk * 128:(chunk + 1) * 128, 128 * k:128 * (k + 1)]
        wts[i] = blk.T.astype(np.float32)
    biasT = np.asarray(bias, np.float32).reshape(2, 128).T.copy()
    return wts, biasT


def _prep_x(x: np.ndarray) -> np.ndarray:
    """(32,128,56,56) -> host-padded (8, 4, 128, 58*58) with zero ring."""
    x = np.asarray(x, np.float32).reshape(B, C_IN, H, W)
    xp = np.zeros((B, C_IN, HP, HP), np.float32)
    xp[:, :, 1:1 + H, 1:1 + W] = x
    return xp.reshape(N_CORES, B_LOC, C_IN, HP * HP)


def kernel(x: np.ndarray, twiddle: np.ndarray, bias: np.ndarray) -> np.ndarray:
    wts, biasT = _prep_params(twiddle, bias)
    runner = _get_runner(1)
    xsh = _prep_x(x)
    in_maps = [{"xs": xsh[c], "wts": wts, "biasT": biasT} for c in range(N_CORES)]
    res = runner(runner.pack(in_maps))
    out = np.stack([res[c]["ys"] for c in range(N_CORES)])  # (8,4,2,128,3136)
    return out.reshape(B, C_OUT, H, W)


# ---- inline copy of the reusable jitted runner (kernel.py self-contained) --
import sys as _sys
import types as _types

_BASS_EXEC_SRC = '''
import numpy as np
import jax
from jax.sharding import Mesh, PartitionSpec
from jax.experimental.shard_map import shard_map

import concourse.mybir as mybir
from concourse.bass2jax import _bass_exec_p, partition_id_tensor, install_neuronx_cc_hook


class BassRunner:
    def __init__(self, nc, n_cores=8):
        install_neuronx_cc_hook()
        assert nc.is_finalized()
        self.nc = nc
        self.n_cores = n_cores
        partition_name = nc.partition_id_tensor.name if nc.partition_id_tensor else None

        in_names, out_names, out_avals, zero_outs = [], [], [], []
        for alloc in nc.m.functions[0].allocations:
            if not isinstance(alloc, mybir.MemoryLocationSet):
                continue
            name = alloc.memorylocations[0].name
            if alloc.kind == "ExternalInput":
                if name != partition_name:
                    in_names.append(name)
            elif alloc.kind == "ExternalOutput":
                out_names.append(name)
                shape = tuple(alloc.tensor_shape)
                dtype = mybir.dt.np(alloc.dtype)
                out_avals.append(jax.core.ShapedArray(shape, dtype))
                zero_outs.append(np.zeros(shape, dtype))
        self.n_params = len(in_names)
        n_outs = len(out_avals)
        self.in_names = list(in_names)
        self.out_names = out_names
        self.out_avals = out_avals
        self.zero_outs = zero_outs
        all_in_names = in_names + out_names
        if partition_name is not None:
            all_in_names.append(partition_name)

        donate = tuple(range(self.n_params, self.n_params + n_outs))

        def _body(*args):
            operands = list(args)
            if partition_name is not None:
                operands.append(partition_id_tensor())
            outs = _bass_exec_p.bind(
                *operands,
                out_avals=tuple(out_avals),
                in_names=tuple(all_in_names),
                out_names=tuple(out_names),
                lowering_input_output_aliases=(),
                sim_require_finite=True,
                sim_require_nnan=True,
                nc=nc,
            )
            return tuple(outs)

        devices = jax.devices()[:n_cores]
        mesh = Mesh(np.asarray(devices), ("core",))
        self._mesh = mesh
        self._zeros_fn = None
        in_specs = (PartitionSpec("core"),) * (self.n_params + n_outs)
        out_specs = (PartitionSpec("core"),) * len(out_names)
        self._fn = jax.jit(
            shard_map(_body, mesh=mesh, in_specs=in_specs, out_specs=out_specs,
                      check_rep=False),
            donate_argnums=donate, keep_unused=True,
        )


    def pack_device(self, in_maps):
        """device_put the packed inputs once; reuse across calls."""
        import jax.numpy as jnp
        from jax.sharding import NamedSharding
        concat = self.pack(in_maps)
        sh = NamedSharding(self._mesh, PartitionSpec("core"))
        return [jax.device_put(a, sh) for a in concat]

    def zeros_device(self):
        if self._zeros_fn is None:
            import jax.numpy as jnp
            from jax.sharding import NamedSharding
            sh = NamedSharding(self._mesh, PartitionSpec("core"))
            shapes = [(self.n_cores * z.shape[0], *z.shape[1:]) for z in self.zero_outs]
            dts = [z.dtype for z in self.zero_outs]

            def _mk():
                return tuple(jnp.zeros(s, d) for s, d in zip(shapes, dts))
            self._zeros_fn = jax.jit(_mk, out_shardings=tuple([sh] * len(shapes)))
        return self._zeros_fn()

    def call_device(self, concat_in_dev):
        """Device-resident call: returns raw jax output arrays."""
        zeros = self.zeros_device()
        return self._fn(*concat_in_dev, *zeros)

    def pack(self, in_maps):
        per_core = [[np.asarray(m[name]) for name in self.in_names] for m in in_maps]
        return [
            np.concatenate([per_core[c][i] for c in range(self.n_cores)], axis=0)
            for i in range(self.n_params)
        ]

    def __call__(self, concat_in, raw=False):
        concat_zeros = [
            np.zeros((self.n_cores * z.shape[0], *z.shape[1:]), z.dtype)
            for z in self.zero_outs
        ]
        out_arrs = self._fn(*concat_in, *concat_zeros)
        if raw:
            return out_arrs
        return [
            {
                name: np.asarray(out_arrs[i]).reshape(
                    self.n_cores, *self.out_avals[i].shape)[c]
                for i, name in enumerate(self.out_names)
            }
            for c in range(self.n_cores)
        ]
'''

_mod = _types.ModuleType("bass_exec_inline")
exec(compile(_BASS_EXEC_SRC, "bass_exec_inline", "exec"), _mod.__dict__)
_sys.modules["bass_exec_inline"] = _mod


# revision 19
# speedup vs baseline: 1.1882x; 1.1882x over previous
"""DeBut 2D conv (32,128,56,56) -> (32,256,56,56) on 8 axon TRN2 NeuronCores.

The butterfly product W3@W2@W1 composes to a block-diagonal (256,1152) matrix
with 32 blocks of (8 out x 36 in). Mapped onto conv weights, output channels
0-127 depend only on kernel positions 0-4 and channels 128-255 only on 4-8.
So the conv is 10 accumulating K=128 f32r matmuls per spatial strip instead
of 18 dense ones.

Per-core layout (batch sharded 4 images/core):
  x image -> SBUF padded (128, 58*58) float32r (zero ring = conv padding)
  7 strips of 8 output rows; moving operand = contiguous 464-col span of the
  padded image; psum (128, 8, 58); drain cols 0:56 with bias via DVE
  tensor_scalar_add (also the only sem lane PE waits on - f32r matmuls can
  encode just one sync wait).
"""
import numpy as np

# ---- problem constants (hardcoded; kernel.py must be self-contained) ----
B, C_IN, H, W = 32, 128, 56, 56
C_OUT = 256
KS = 3
N_CORES = 8
B_LOC = B // N_CORES          # 4 images per core
HP = H + 2                    # 58
PADDED = HP * HP + 8          # 3372, slack for junk-column overreads
STRIP_ROWS = 8
N_STRIPS = H // STRIP_ROWS    # 7
N_MM = STRIP_ROWS * HP        # 464 moving columns per matmul
N_OUT = STRIP_ROWS * W        # 448 valid columns per strip
R_SHAPES = [(768, 1152, 2, 3, 1), (512, 768, 2, 3, 2), (256, 512, 2, 4, 4)]
# (chunk, kpos) pairs: chunk0 -> kpos 0..4, chunk1 -> kpos 4..8
CHUNK_KPOS = [(0, k) for k in range(5)] + [(1, k) for k in range(4, 9)]

_RUNNER = None


def _compose_w(twiddle: np.ndarray) -> np.ndarray:
    """Compose butterfly factors into the dense (256, 1152) matrix (float64)."""
    W_full = None
    temp = 0
    for (osz, isz, row, col, diag) in R_SHAPES:
        npar = col * osz
        nb = isz // (col * diag)
        t = twiddle[temp:temp + npar].astype(np.float64)
        t = t.reshape(nb, diag, row, col).transpose(0, 2, 3, 1)  # (n, r, c, d)
        temp += npar
        Ws = np.zeros((osz, isz), np.float64)
        # out index n*row*diag + r*diag + d ; in index n*col*diag + c*diag + d
        for d in range(diag):
            for r in range(row):
                for c in range(col):
                    out_idx = np.arange(nb) * row * diag + r * diag + d
                    in_idx = np.arange(nb) * col * diag + c * diag + d
                    Ws[out_idx, in_idx] = t[:, r, c, d]
        W_full = Ws if W_full is None else Ws @ W_full
    return W_full  # (256, 1152)


def _build_nc(repeat: int = 1, trace_sim: bool = False, mode: str = 'full'):
    import concourse.bass as bass  # noqa: F401
    from concourse import bacc
    import concourse.mybir as mybir
    from concourse.tile import TileContext

    f32 = mybir.dt.float32
    f32r = mybir.dt.float32r

    nc = bacc.Bacc("TRN2", target_bir_lowering=False, debug=False,
                   num_devices=N_CORES)
    # xs/wts are declared float32r: same 4-byte layout (numpy float32 binds),
    # lets plain HWDGE DMAs feed the f32r matmuls with no cast pass.
    # xs arrives host-padded to 58x58 (zero ring included) so the in-DMA is
    # one contiguous 13.4KB run per partition.
    xs = nc.declare_dram_parameter("xs", [B_LOC, C_IN, HP * HP], f32r,
                                   isOutput=False)
    wts = nc.declare_dram_parameter("wts", [10, C_IN, 128], f32r, isOutput=False)
    biasT = nc.declare_dram_parameter("biasT", [128, 2], f32, isOutput=False)
    ys = nc.declare_dram_parameter("ys", [B_LOC, 2, 128, H * W], f32, isOutput=True)

    with TileContext(nc, trace_sim=trace_sim) as tc:
        with tc.tile_pool(name="sbuf", bufs=1) as cpool, \
             tc.tile_pool(name="outp", bufs=2) as opool, \
             tc.tile_pool(name="psum", bufs=6, space="PSUM") as ppool:
            # persistent padded-image slots; 8-col slack zeroed once below
            N_XPAD = 3
            xpads = [cpool.tile([C_IN, PADDED], f32r, tag=f"xpad{i}",
                                name=f"xpad{i}")
                     for i in range(N_XPAD)]
            zrow = cpool.tile([C_IN, 8], f32, tag="zrow")
            nc.vector.memset(zrow[:], 0.0)
            for xp in xpads:
                nc.vector.tensor_copy(xp[:, HP * HP:], zrow[:])   # slack
            # first image on the SP HWDGE queue, weights+bias on the ACT HWDGE
            # queue - they land in parallel
            SPLIT0 = 35 * HP
            if mode not in ('no_in', 'pe_only'):
                nc.sync.dma_start(out=xpads[0][:, :SPLIT0], in_=xs[0, :, :SPLIT0])
                nc.sync.dma_start(out=xpads[0][:, SPLIT0:HP * HP],
                                  in_=xs[0, :, SPLIT0:])
            wt_r = cpool.tile([C_IN, 10, 128], f32r, tag="wtr")
            nc.scalar.dma_start(out=wt_r[:], in_=wts.ap().rearrange("i c m -> c i m"))
            bias_sb = cpool.tile([128, 2], f32, tag="bias")
            nc.scalar.dma_start(out=bias_sb[:], in_=biasT.ap())

            # out-DMA split points: after strip 3 (cols 0:1792), after strip 5
            # (1792:2688), after strip 6 (2688:3136) - shrinks the tail
            OUT_SPLITS = {3: (0, 4 * N_OUT), 5: (4 * N_OUT, 6 * N_OUT),
                          6: (6 * N_OUT, 7 * N_OUT)}

            def body():
                for b in range(B_LOC):
                    if (b > 0 or repeat > 1) and mode not in ('no_in', 'pe_only'):
                        nc.sync.dma_start(out=xpads[b % N_XPAD][:, :HP * HP],
                                          in_=xs[b])

                    out_sb = opool.tile([128, 2, H * W], f32, tag="outsb")
                    for s in range(N_STRIPS):
                        for chunk in range(2):
                            ps = ppool.tile([128, STRIP_ROWS, HP], f32, tag="ps")
                            for kidx in range(5):
                                _, k = CHUNK_KPOS[chunk * 5 + kidx]
                                di, dj = k // 3, k % 3
                                start = (STRIP_ROWS * s + di) * HP + dj
                                nc.tensor.matmul(
                                    ps[:],
                                    wt_r[:, chunk * 5 + kidx, :],
                                    xpads[b % N_XPAD][:, start:start + N_MM],
                                    start=(kidx == 0), stop=(kidx == 4),
                                )
                            nc.vector.tensor_scalar_add(
                                out_sb[:, chunk, s * N_OUT:(s + 1) * N_OUT]
                                .rearrange("p (r w) -> p r w", w=W),
                                ps[:, :, 0:W],
                                bias_sb[:, chunk:chunk + 1],
                            )
                        if s in OUT_SPLITS:
                            lo, hi = OUT_SPLITS[s]
                            if mode in ('no_out', 'pe_only'):
                                if s == 6:
                                    nc.scalar.dma_start(out=ys[b, 0, :, :16],
                                                        in_=out_sb[:, 0, :16])
                            else:
                                nc.scalar.dma_start(
                                    out=ys[b, :, :, lo:hi]
                                    .rearrange("c2 p hw -> p c2 hw"),
                                    in_=out_sb[:, :, lo:hi])

            if repeat == 1:
                body()
            else:
                with tc.For_i(0, repeat, 1):
                    body()
    nc.finalize()
    return nc


def _get_runner(repeat: int = 1):
    global _RUNNER
    if _RUNNER is None or _RUNNER[0] != repeat:
        from bass_exec_inline import BassRunner
        nc = _build_nc(repeat)
        _RUNNER = (repeat, BassRunner(nc, n_cores=N_CORES))
    return _RUNNER[1]


def _prep_params(twiddle: np.ndarray, bias: np.ndarray):
    W_dense = _compose_w(np.asarray(twiddle))
    wts = np.empty((10, C_IN, 128), np.float32)
    for i, (chunk, k) in enumerate(CHUNK_KPOS):
        blk = W_dense[chunk * 128:(chunk + 1) * 128, 128 * k:128 * (k + 1)]
        wts[i] = blk.T.astype(np.float32)
    biasT = np.asarray(bias, np.float32).reshape(2, 128).T.copy()
    return wts, biasT


def _prep_x(x: np.ndarray) -> np.ndarray:
    """(32,128,56,56) -> host-padded (8, 4, 128, 58*58) with zero ring."""
    x = np.asarray(x, np.float32).reshape(B, C_IN, H, W)
    xp = np.zeros((B, C_IN, HP, HP), np.float32)
    xp[:, :, 1:1 + H, 1:1 + W] = x
    return xp.reshape(N_CORES, B_LOC, C_IN, HP * HP)


def kernel(x: np.ndarray, twiddle: np.ndarray, bias: np.ndarray) -> np.ndarray:
    wts, biasT = _prep_params(twiddle, bias)
    runner = _get_runner(1)
    xsh = _prep_x(x)
    in_maps = [{"xs": xsh[c], "wts": wts, "biasT": biasT} for c in range(N_CORES)]
    res = runner(runner.pack(in_maps))
    out = np.stack([res[c]["ys"] for c in range(N_CORES)])  # (8,4,2,128,3136)
    return out.reshape(B, C_OUT, H, W)


# ---- inline copy of the reusable jitted runner (kernel.py self-contained) --
import sys as _sys
import types as _types

_BASS_EXEC_SRC = '''
import numpy as np
import jax
from jax.sharding import Mesh, PartitionSpec
from jax.experimental.shard_map import shard_map

import concourse.mybir as mybir
from concourse.bass2jax import _bass_exec_p, partition_id_tensor, install_neuronx_cc_hook


class BassRunner:
    def __init__(self, nc, n_cores=8):
        install_neuronx_cc_hook()
        assert nc.is_finalized()
        self.nc = nc
        self.n_cores = n_cores
        partition_name = nc.partition_id_tensor.name if nc.partition_id_tensor else None

        in_names, out_names, out_avals, zero_outs = [], [], [], []
        for alloc in nc.m.functions[0].allocations:
            if not isinstance(alloc, mybir.MemoryLocationSet):
                continue
            name = alloc.memorylocations[0].name
            if alloc.kind == "ExternalInput":
                if name != partition_name:
                    in_names.append(name)
            elif alloc.kind == "ExternalOutput":
                out_names.append(name)
                shape = tuple(alloc.tensor_shape)
                dtype = mybir.dt.np(alloc.dtype)
                out_avals.append(jax.core.ShapedArray(shape, dtype))
                zero_outs.append(np.zeros(shape, dtype))
        self.n_params = len(in_names)
        n_outs = len(out_avals)
        self.in_names = list(in_names)
        self.out_names = out_names
        self.out_avals = out_avals
        self.zero_outs = zero_outs
        all_in_names = in_names + out_names
        if partition_name is not None:
            all_in_names.append(partition_name)

        donate = tuple(range(self.n_params, self.n_params + n_outs))

        def _body(*args):
            operands = list(args)
            if partition_name is not None:
                operands.append(partition_id_tensor())
            outs = _bass_exec_p.bind(
                *operands,
                out_avals=tuple(out_avals),
                in_names=tuple(all_in_names),
                out_names=tuple(out_names),
                lowering_input_output_aliases=(),
                sim_require_finite=True,
                sim_require_nnan=True,
                nc=nc,
            )
            return tuple(outs)

        devices = jax.devices()[:n_cores]
        mesh = Mesh(np.asarray(devices), ("core",))
        self._mesh = mesh
        self._zeros_fn = None
        in_specs = (PartitionSpec("core"),) * (self.n_params + n_outs)
        out_specs = (PartitionSpec("core"),) * len(out_names)
        self._fn = jax.jit(
            shard_map(_body, mesh=mesh, in_specs=in_specs, out_specs=out_specs,
                      check_rep=False),
            donate_argnums=donate, keep_unused=True,
        )


    def pack_device(self, in_maps):
        """device_put the packed inputs once; reuse across calls."""
        import jax.numpy as jnp
        from jax.sharding import NamedSharding
        concat = self.pack(in_maps)
        sh = NamedSharding(self._mesh, PartitionSpec("core"))
        return [jax.device_put(a, sh) for a in concat]

    def zeros_device(self):
        if self._zeros_fn is None:
            import jax.numpy as jnp
            from jax.sharding import NamedSharding
            sh = NamedSharding(self._mesh, PartitionSpec("core"))
            shapes = [(self.n_cores * z.shape[0], *z.shape[1:]) for z in self.zero_outs]
            dts = [z.dtype for z in self.zero_outs]

            def _mk():
                return tuple(jnp.zeros(s, d) for s, d in zip(shapes, dts))
            self._zeros_fn = jax.jit(_mk, out_shardings=tuple([sh] * len(shapes)))
        return self._zeros_fn()

    def call_device(self, concat_in_dev):
        """Device-resident call: returns raw jax output arrays."""
        zeros = self.zeros_device()
        return self._fn(*concat_in_dev, *zeros)

    def pack(self, in_maps):
        per_core = [[np.asarray(m[name]) for name in self.in_names] for m in in_maps]
        return [
            np.concatenate([per_core[c][i] for c in range(self.n_cores)], axis=0)
            for i in range(self.n_params)
        ]

    def __call__(self, concat_in, raw=False):
        concat_zeros = [
            np.zeros((self.n_cores * z.shape[0], *z.shape[1:]), z.dtype)
            for z in self.zero_outs
        ]
        out_arrs = self._fn(*concat_in, *concat_zeros)
        if raw:
            return out_arrs
        return [
            {
                name: np.asarray(out_arrs[i]).reshape(
                    self.n_cores, *self.out_avals[i].shape)[c]
                for i, name in enumerate(self.out_names)
            }
            for c in range(self.n_cores)
        ]
'''

_mod = _types.ModuleType("bass_exec_inline")
exec(compile(_BASS_EXEC_SRC, "bass_exec_inline", "exec"), _mod.__dict__)
_sys.modules["bass_exec_inline"] = _mod
